# revision 1
# baseline (speedup 1.0000x reference)
"""Trainium2 Bass kernel for the sparse-conv network (nn_ExampleNet).

Pipeline (per batch image): scatter 200k sparse voxel features into a dense
[256,256,32] grid, SparseConv(32->64) + 2x SubMConv(64) with an active-site
mask, SparseConvTranspose(64, stride 2), dense 3x3 VALID conv -> [511,511,64].

Strategy: 8-way SPMD (4 batches x 2 row-halves). Host does the sparse->dense
scatter, mask dilations and data layout; each NeuronCore runs the fused
5-conv stack on its half-image in 16-row chunks, convs expressed as
shifted-window fp32r matmuls (taps packed in pairs across 128 partitions).
"""
from contextlib import ExitStack

import numpy as np
import ml_dtypes

import concourse.bacc as bacc
import concourse.mybir as mybir
import concourse.tile as tile
from concourse.bass_utils import run_bass_kernel_spmd

F32 = mybir.dt.float32
F32R = mybir.dt.float32r
BF16 = mybir.dt.bfloat16
RELU = mybir.ActivationFunctionType.Relu

B, H, W = 4, 256, 256
WP = W + 2          # padded width for x/h1-3 stores
CH = 16             # output rows per chunk
NCH = 16            # chunks per core
PITCH4 = 516        # h4 row pitch (513 cols + pad)
RXS = 138           # x slab rows
RM1 = 136           # mask1 slab rows
RM4 = 260           # mask4 slab rows

BLOCKS1 = [(0, 0), (0, 2), (2, 0), (2, 2)]
BLOCKS6 = [(d, x) for d in range(3) for x in (0, 2)]

_CACHE = {}


def _host_prep(features, coors, w1, b1, w2, b2, w3, b3, wt, bt, w5, b5):
    f32 = np.float32
    bi, yi, xi = coors[:, 0], coors[:, 1], coors[:, 2]
    flat = (bi.astype(np.int64) * H + yi) * W + xi
    dense = np.zeros((B * H * W, 32), f32)
    for c in range(32):
        dense[:, c] = np.bincount(flat, weights=features[:, c],
                                  minlength=B * H * W)
    dense = dense.reshape(B, H, W, 32)
    occ = np.bincount(flat, minlength=B * H * W).reshape(B, H, W) > 0
    m0p = np.zeros((B, H + 2, W + 2), bool)
    m0p[:, 1:-1, 1:-1] = occ
    m1 = np.zeros((B, H, W), bool)
    for dy in range(3):
        for dx in range(3):
            m1 |= m0p[:, dy:dy + H, dx:dx + W]
    m4 = np.zeros((B, 2 * H + 1, 2 * W + 1), bool)
    for dy in range(3):
        for dx in range(3):
            m4[:, dy:dy + 2 * H - 1:2, dx:dx + 2 * W - 1:2] |= m1

    wt_eff = wt[::-1, ::-1]  # jax conv_transpose applies the flipped kernel
    z32 = np.zeros((32, 64), f32)
    z64 = np.zeros((64, 64), f32)
    wc1 = np.zeros((128, 4, 64), f32)
    for i, (dy, dx) in enumerate(BLOCKS1):
        for g, (jy, jx) in enumerate([(0, 0), (0, 1), (1, 0), (1, 1)]):
            tap = w1[dy + jy, dx + jx] if (dy + jy < 3 and dx + jx < 3) else z32
            wc1[32 * g:32 * g + 32, i] = tap

    def mk6(w):
        out = np.zeros((128, 6, 64), f32)
        for i, (dy, dxb) in enumerate(BLOCKS6):
            out[0:64, i] = w[dy, dxb]
            out[64:128, i] = w[dy, dxb + 1] if dxb + 1 < 3 else z64
        return out

    wc2, wc3, wc5 = mk6(w2), mk6(w3), mk6(w5)
    wct = np.zeros((128, 6, 64), f32)
    wct[0:64, 0], wct[64:128, 0] = wt_eff[0, 2], wt_eff[0, 0]
    wct[0:64, 1], wct[64:128, 1] = wt_eff[2, 2], wt_eff[2, 0]
    wct[0:64, 2] = wt_eff[0, 1]
    wct[0:64, 3] = wt_eff[2, 1]
    wct[0:64, 4], wct[64:128, 4] = wt_eff[1, 2], wt_eff[1, 0]
    wct[0:64, 5] = wt_eff[1, 1]
    biases = np.stack([b1, b2, b3, bt, b5], 1).astype(f32)

    in_maps = []
    for core in range(8):
        b, half = core // 2, core % 2
        A0 = 0 if half == 0 else 128
        U0 = 2 * A0
        xs = np.zeros((32, RXS, WP), f32)
        lo, hi = max(0, A0 - 4), min(H, A0 - 4 + RXS)
        xs[:, lo - (A0 - 4):hi - (A0 - 4), 1:W + 1] = \
            dense[b, lo:hi].transpose(2, 0, 1)
        ms = np.zeros((RM1, WP), ml_dtypes.bfloat16)
        lo, hi = max(0, A0 - 3), min(H, A0 - 3 + RM1)
        ms[lo - (A0 - 3):hi - (A0 - 3), 1:W + 1] = m1[b, lo:hi]
        m4s = np.zeros((RM4, PITCH4), ml_dtypes.bfloat16)
        lo, hi = max(0, U0), min(2 * H + 1, U0 + RM4)
        m4s[lo - U0:hi - U0, :2 * W + 1] = m4[b, lo:hi]
        in_maps.append(dict(
            xs=np.ascontiguousarray(xs.reshape(32, -1)),
            ms=np.ascontiguousarray(np.broadcast_to(ms[None], (64, RM1, WP))),
            m4s=np.ascontiguousarray(
                np.broadcast_to(m4s[None], (64, RM4, PITCH4))),
            wc1=wc1, wc2=wc2, wc3=wc3, wct=wct, wc5=wc5, biases=biases,
        ))
    return in_maps


def _build_program():
    nc = bacc.Bacc("TRN2", target_bir_lowering=False, debug=False,
                   enable_asserts=True, num_devices=8)

    xs_d = nc.dram_tensor("xs", [32, RXS * WP], F32R, kind="ExternalInput").ap()
    ms_d = nc.dram_tensor("ms", [64, RM1, WP], BF16, kind="ExternalInput").ap()
    m4_d = nc.dram_tensor("m4s", [64, RM4, PITCH4], BF16,
                          kind="ExternalInput").ap()
    wc1_d = nc.dram_tensor("wc1", [128, 4, 64], F32R, kind="ExternalInput").ap()
    wc2_d = nc.dram_tensor("wc2", [128, 6, 64], F32R, kind="ExternalInput").ap()
    wc3_d = nc.dram_tensor("wc3", [128, 6, 64], F32R, kind="ExternalInput").ap()
    wct_d = nc.dram_tensor("wct", [128, 6, 64], F32R, kind="ExternalInput").ap()
    wc5_d = nc.dram_tensor("wc5", [128, 6, 64], F32R, kind="ExternalInput").ap()
    bias_d = nc.dram_tensor("biases", [64, 5], F32, kind="ExternalInput").ap()
    out_d = nc.dram_tensor("out", [64, 256 * 511], F32,
                           kind="ExternalOutput").ap()

    with tile.TileContext(nc) as tc, ExitStack() as ctx:
        wp = ctx.enter_context(tc.tile_pool(name="wp", bufs=1))
        xp = ctx.enter_context(tc.tile_pool(name="xp", bufs=2))
        mp = ctx.enter_context(tc.tile_pool(name="mp", bufs=2))
        hp = ctx.enter_context(tc.tile_pool(name="hp", bufs=1))
        pp = ctx.enter_context(tc.tile_pool(name="pp", bufs=2, space="PSUM"))
        op = ctx.enter_context(tc.tile_pool(name="op", bufs=4))

        w1t = wp.tile([128, 4, 64], F32R, name="w1t")
        w2t = wp.tile([128, 6, 64], F32R, name="w2t")
        w3t = wp.tile([128, 6, 64], F32R, name="w3t")
        wtt = wp.tile([128, 6, 64], F32R, name="wtt")
        w5t = wp.tile([128, 6, 64], F32R, name="w5t")
        bt = wp.tile([64, 5], F32, name="bt")
        nc.sync.dma_start(w1t[:], wc1_d[:])
        nc.sync.dma_start(w2t[:], wc2_d[:])
        nc.sync.dma_start(w3t[:], wc3_d[:])
        nc.sync.dma_start(wtt[:], wct_d[:])
        nc.sync.dma_start(w5t[:], wc5_d[:])
        nc.sync.dma_start(bt[:], bias_d[:])

        def conv_layer(inp, wt_, blocks, nrows, bias_ap, m_ch, moff, h_out):
            for j in range(0, nrows, 2):
                pc = pp.tile([64, 2, 256], F32, name="pc", tag="pc")
                for i, (dy, dx) in enumerate(blocks):
                    nc.tensor.matmul(
                        pc[:], wt_[:, i, :], inp[:, j + dy:j + dy + 2,
                                                 dx:dx + 256],
                        start=(i == 0), stop=(i == len(blocks) - 1))
                dst = h_out[0:64, j:j + 2, 1:257]
                nc.scalar.activation(dst, pc[:], RELU, bias=bias_ap)
                nc.vector.tensor_mul(dst, dst,
                                     m_ch[0:64, j + moff:j + moff + 2, 1:257])
                nc.sync.dma_start(h_out[64:128, j:j + 2, 0:256], dst)

        for c in range(NCH):
            x_ch = xp.tile([128, CH, WP], F32R, name="x_ch", tag="x")
            for g, s in enumerate([0, 1, WP, WP + 1]):
                base = (8 * c) * WP + s
                src = xs_d[:, base:base + CH * WP].rearrange(
                    "p (r c) -> p r c", c=WP)
                nc.sync.dma_start(x_ch[32 * g:32 * g + 32, :, :], src)
            m1_ch = mp.tile([64, 14, WP], BF16, name="m1_ch", tag="m1")
            nc.sync.dma_start(m1_ch[:], ms_d[:, 8 * c:8 * c + 14, :])
            m4_ch = mp.tile([64, 18, PITCH4], BF16, name="m4_ch", tag="m4")
            nc.sync.dma_start(m4_ch[:], m4_d[:, 16 * c:16 * c + 18, :])

            h1 = hp.tile([128, 14, WP], F32R, name="h1", tag="h1")
            h2 = hp.tile([128, 12, WP], F32R, name="h2", tag="h2")
            h3 = hp.tile([128, 10, WP], F32R, name="h3", tag="h3")
            h4 = hp.tile([128, 18, PITCH4], F32R, name="h4", tag="h4")
            for h_ in (h1, h2, h3):
                nc.gpsimd.memset(h_[0:128, :, 0:1].bitcast(F32), 0)
                nc.gpsimd.memset(h_[0:128, :, 257:258].bitcast(F32), 0)
                nc.gpsimd.memset(h_[64:128, :, 256:257].bitcast(F32), 0)
            nc.gpsimd.memset(h4[64:128, :, 512:513].bitcast(F32), 0)
            nc.gpsimd.memset(h4[0:128, :, 513:516].bitcast(F32), 0)

            conv_layer(x_ch, w1t, BLOCKS1, 14, bt[:, 0:1], m1_ch, 0, h1)
            conv_layer(h1, w2t, BLOCKS6, 12, bt[:, 1:2], m1_ch, 1, h2)
            conv_layer(h2, w3t, BLOCKS6, 10, bt[:, 2:3], m1_ch, 2, h3)

            for la in range(9):
                p00 = pp.tile([64, 258], F32, name="p00", tag="pT")
                nc.tensor.matmul(p00[:], wtt[:, 0, :], h3[:, la + 1, 0:258],
                                 start=True, stop=False)
                nc.tensor.matmul(p00[:], wtt[:, 1, :], h3[:, la, 0:258],
                                 start=False, stop=True)
                p01 = pp.tile([64, 256], F32, name="p01", tag="pT")
                nc.tensor.matmul(p01[:], wtt[:, 2, :], h3[:, la + 1, 1:257],
                                 start=True, stop=False)
                nc.tensor.matmul(p01[:], wtt[:, 3, :], h3[:, la, 1:257],
                                 start=False, stop=True)
                p10 = pp.tile([64, 258], F32, name="p10", tag="pT")
                nc.tensor.matmul(p10[:], wtt[:, 4, :], h3[:, la + 1, 0:258],
                                 start=True, stop=True)
                p11 = pp.tile([64, 256], F32, name="p11", tag="pT")
                nc.tensor.matmul(p11[:], wtt[:, 5, :], h3[:, la + 1, 1:257],
                                 start=True, stop=True)
                for pu, (pe, po) in enumerate([(p00, p01), (p10, p11)]):
                    u = 2 * la + pu
                    de = h4[0:64, u, 0:513:2]
                    do = h4[0:64, u, 1:513:2]
                    nc.scalar.activation(de, pe[:, 0:257], RELU, bias=bt[:, 3:4])
                    nc.scalar.activation(do, po[:], RELU, bias=bt[:, 3:4])
                    nc.vector.tensor_mul(de, de, m4_ch[0:64, u, 0:513:2])
                    nc.vector.tensor_mul(do, do, m4_ch[0:64, u, 1:513:2])
                    nc.sync.dma_start(h4[64:128, u, 0:512], h4[0:64, u, 1:513])

            for jo in range(CH):
                p5 = pp.tile([64, 512], F32, name="p5", tag="p5")
                for i, (dy, dxb) in enumerate(BLOCKS6):
                    nc.tensor.matmul(p5[:], w5t[:, i, :],
                                     h4[:, jo + dy, dxb:dxb + 512],
                                     start=(i == 0), stop=(i == 5))
                out_sb = op.tile([64, 511], F32, name="out_sb", tag="o")
                nc.scalar.activation(out_sb[:], p5[:, 0:511], RELU,
                                     bias=bt[:, 4:5])
                nc.sync.dma_start(
                    out_d[:, (16 * c + jo) * 511:(16 * c + jo + 1) * 511],
                    out_sb[:])

    nc.compile()
    return nc


def kernel(**inputs):
    features = np.asarray(inputs["features"], np.float32)
    coors = np.asarray(inputs["coors"], np.int32)
    args = [np.asarray(inputs[k], np.float32) for k in
            ("w1", "b1", "w2", "b2", "w3", "b3", "wt", "bt", "w5", "b5")]
    in_maps = _host_prep(features, coors, *args)
    if "nc" not in _CACHE:
        _CACHE["nc"] = _build_program()
    res = run_bass_kernel_spmd(_CACHE["nc"], in_maps,
                               core_ids=list(range(8)), trace=False)
    full = np.zeros((B, 511, 511, 64), np.float32)
    for core in range(8):
        o = res.results[core]["out"].reshape(64, 256, 511)
        b, half = core // 2, core % 2
        if half == 0:
            full[b, 0:256] = o.transpose(1, 2, 0)
        else:
            full[b, 256:511] = o[:, 0:255].transpose(1, 2, 0)
    return full
